# revision 27
# baseline (speedup 1.0000x reference)
"""GQA attention (B=2, S=2048, D=1024, 16 q heads / 4 kv heads, RoPE, causal)
on 8 NeuronCores.

Sharding: core c handles batch b = c // 4 and kv head kv = c % 4 (its 4 q
heads). Each core computes x[b] @ {wq,wkv} slices, RoPE, causal attention in
transposed [s, q] layout (probs feed P@V directly as the matmul moving
operand), then its 256-row slice of w_o. The 4 partial [S, D] outputs per
batch are summed on the host.

v2 (from trace analysis of the fp32r baseline, 409us):
- bf16 matmul inputs everywhere (PSUM accumulation stays fp32). Same PE
  stream rate as fp32r, but 4x cheaper LDWEIGHTS (FWL), half the DMA bytes
  and SBUF footprint.
- exp merged across the two head-halves per block: scores for heads e/o of
  a pair land in one 2-bank PSUM tile [128, 2, 512], one ACTIVATE covers
  both (halves the ~300ns/call ACT overhead).
- softmax normalization: wide ops only. Sum rows are copied to partitions
  {0, 64} of one staging tile, ONE wide reciprocal, broadcast via two K=1
  matmuls into one PSUM tile, then two DVE muls produce the normalized
  attention tile. (Baseline burned 3.3us per 1-lane reciprocal and ~5us of
  serialized tail per pair that stalled the PE cold.)
- V transposed via the DMA XBAR (dma_start_transpose, bf16) instead of PE
  transpose-mode matmuls: no PSUM round-trip, no PE stall.
- w_o for q-tile j-1 is emitted interleaved into attention of tile j, so
  the PE fills ACT-bound gaps and never waits on the normalization tail.
- DMA: rope tables go on the ACT queue in parallel with weights+x on the
  Sync queue; x loads are 4 big DMAs (one per q tile); outputs are written
  bf16, 4 merged DMAs per tile.
"""
import numpy as np

import concourse.bass as bass
import concourse.mybir as mybir
import concourse.tile as tile
from concourse import bacc
from concourse.bass_utils import run_bass_kernel_spmd

F32 = mybir.dt.float32
F32R = mybir.dt.float32r
BF16 = mybir.dt.bfloat16
EXP = mybir.ActivationFunctionType.Exp

B = 2
S_FULL = 2048
D = 1024
DK = 64
NH = 16
NKV = 4
GROUP = 4
ROPE_BASE = 10000.0
N_CORES = 8
NT = 512          # q-tile (matmul moving free dim)
SB = 128          # s-block (matmul contraction block)


def build_nc(S=S_FULL):
    nj = S // NT          # q tiles
    nsb = S // SB         # s blocks
    nck = D // 128        # contraction chunks over D

    nc = bacc.Bacc("TRN2", target_bir_lowering=False, debug=False,
                   num_devices=N_CORES)

    xT = nc.dram_tensor("xT", [D, S], BF16, kind="ExternalInput")
    wq = nc.dram_tensor("wq", [D, GROUP * DK], BF16, kind="ExternalInput")
    wkv = nc.dram_tensor("wkv", [D, 2 * DK], BF16, kind="ExternalInput")
    wo = nc.dram_tensor("wo", [GROUP * DK, D], BF16, kind="ExternalInput")
    cos4 = nc.dram_tensor("cos4", [128, S], F32, kind="ExternalInput")
    sin4 = nc.dram_tensor("sin4", [128, S], F32, kind="ExternalInput")
    out_part = nc.dram_tensor("out_part", [S, D], BF16, kind="ExternalOutput")

    with tile.TileContext(nc) as tc, nc.allow_low_precision(
            reason="bf16 matmul inputs with fp32 PSUM accumulation keep the "
                   "max rel err ~1e-3, far inside this problem's 2e-2 gate"):
        with (
            tc.tile_pool(name="sb_const", bufs=1) as sb_const,
            tc.tile_pool(name="sb_xt", bufs=nj) as sb_xt,
            tc.tile_pool(name="sb_w", bufs=1) as sb_w,
            tc.tile_pool(name="sb_qt", bufs=2 * nj) as sb_qt,
            tc.tile_pool(name="sb_kt", bufs=nj) as sb_kt,
            tc.tile_pool(name="sb_va", bufs=nsb) as sb_va,
            tc.tile_pool(name="sb_at", bufs=2 * nj) as sb_at,
            tc.tile_pool(name="sb_tmp", bufs=6) as sb_tmp,
            tc.tile_pool(name="sb_ex", bufs=4) as sb_ex,
            tc.tile_pool(name="sb_out", bufs=3) as sb_out,
            tc.tile_pool(name="ps_proj", bufs=2, space="PSUM") as ps_proj,
            tc.tile_pool(name="ps_sc", bufs=2, space="PSUM") as ps_sc,
            tc.tile_pool(name="ps_pv", bufs=2, space="PSUM") as ps_pv,
        ):
            # ------------- weights / tables / constants (prefetch) ---------
            # Weights + x on the Sync DMA queue; rope tables ride the ACT
            # queue so both streams land in parallel at startup.
            wkv_sb = sb_w.tile([128, nck, 2 * DK], BF16, tag="wkv")
            nc.sync.dma_start(out=wkv_sb[:],
                              in_=wkv.ap().rearrange("(c p) m -> p c m", p=128))
            wq_sb = sb_w.tile([128, nck, GROUP * DK], BF16, tag="wq")
            nc.sync.dma_start(out=wq_sb[:],
                              in_=wq.ap().rearrange("(c p) m -> p c m", p=128))

            # rope tables arrive per-q-tile so tile 0's RoPE isn't gated on
            # the full 2MB transfer
            cos_sb = sb_const.tile([128, S], F32, tag="cos4")
            sin_sb = sb_const.tile([128, S], F32, tag="sin4")
            for j in range(nj):
                jt = slice(j * NT, (j + 1) * NT)
                nc.scalar.dma_start(out=cos_sb[:, jt], in_=cos4[:, jt])
                nc.scalar.dma_start(out=sin_sb[:, jt], in_=sin4[:, jt])

            xT_ap = xT.ap().rearrange("(c p) s -> p c s", p=128)
            xt = []
            for j in range(nj):
                t = sb_xt.tile([128, nck, NT], BF16, tag="xt")
                if j == 0:
                    # per-chunk DMAs: the first projection matmul starts as
                    # soon as chunk 0 lands instead of after the full 1MB
                    for ck in range(nck):
                        nc.sync.dma_start(
                            out=t[:, ck, :],
                            in_=xT_ap[:, ck, j * NT:(j + 1) * NT])
                else:
                    nc.sync.dma_start(out=t[:],
                                      in_=xT_ap[:, :, j * NT:(j + 1) * NT])
                xt.append(t)

            wo_sb = sb_w.tile([128, 2, D], BF16, tag="wo")
            nc.scalar.dma_start(out=wo_sb[:],
                                in_=wo.ap().rearrange("(c p) e -> p c e", p=128))

            # Broadcast-matmul stationary: row 0 -> out cols 0:64 (head e),
            # row 64 -> out cols 64:128 (head o), other rows zero. One K=65
            # matmul then broadcasts both reciprocal rows at once. (memset
            # can't write float32r; stage f32 then copy-cast)
            maskf = sb_const.tile([65, 128], F32, tag="maskf")
            nc.gpsimd.memset(maskf[:], 0.0)
            nc.gpsimd.memset(maskf[0:1, 0:64], 1.0)
            nc.gpsimd.memset(maskf[64:65, 64:128], 1.0)
            mask65 = sb_const.tile([65, 128], F32R, tag="mask65")
            nc.vector.tensor_copy(mask65[:], maskf[:])
            # Persistent softmax-reciprocal staging tiles: rows 0 and 64 are
            # live (head e / head o reciprocal sums), rows 1:64 stay 1.0
            # forever so the masked broadcast matmul never sees
            # uninitialized data. srr is the f32r-rounded copy the matmul
            # consumes (the BIR verifier requires explicit f32r rounding).
            sr = sb_const.tile([65, NT], F32, tag="sr")
            nc.gpsimd.memset(sr[:], 1.0)
            srr = sb_const.tile([65, NT], F32R, tag="srr")
            nc.vector.tensor_copy(srr[:], sr[:])

            vstage = sb_const.tile([64, S], BF16, tag="vstage")

            # V^T tiles [s=128, 64 v | 1 ones]; ones column preset once
            va = []
            for i in range(nsb):
                t = sb_va.tile([128, 65], BF16, tag="va")
                nc.gpsimd.memset(t[:, 64:65], 1.0)
                va.append(t)

            qt = [[None] * nj for _ in range(2)]   # [pair][j] -> [128, NT]
            kt = [None] * nj                       # [j] -> [128, NT] (dup)
            at = [[None] * nj for _ in range(2)]   # [pair][j] -> [128, NT]

            def proj_j(j):
                jt = slice(j * NT, (j + 1) * NT)
                # ---- kv projection (wkv = [wv | wk]) ----
                pkv = ps_proj.tile([128, NT], F32, tag="proj")
                for ck in range(nck):
                    nc.tensor.matmul(pkv[:], wkv_sb[:, ck, :], xt[j][:, ck, :],
                                     start=(ck == 0), stop=(ck == nck - 1))
                nc.vector.tensor_copy(vstage[:, jt], pkv[0:64, :])
                # ---- K rope on rows 64:128 (muls read PSUM directly:
                # cross-partition-base DVE ops are only legal with a PSUM
                # input) ----
                ktile = sb_kt.tile([128, NT], BF16, tag="kt")
                kt[j] = ktile
                tmp = sb_tmp.tile([128, NT], BF16, tag="rope_tmp")
                nc.vector.tensor_mul(tmp[64:96, :], pkv[96:128, :],
                                     sin_sb[64:96, jt])
                nc.vector.tensor_mul(tmp[96:128, :], pkv[64:96, :],
                                     sin_sb[96:128, jt])
                tmp2 = sb_tmp.tile([128, NT], BF16, tag="rope_tmp2")
                nc.vector.tensor_mul(tmp2[64:128, :], pkv[64:128, :],
                                     cos_sb[64:128, jt])
                nc.gpsimd.tensor_add(ktile[64:128, :], tmp2[64:128, :],
                                     tmp[64:128, :])
                # roped K is needed on both partition halves (score quadrant
                # split); SB->SB partition moves must go through the DMA
                nc.sync.dma_start(out=ktile[0:64, :], in_=ktile[64:128, :])
                # V^T via the DMA transpose XBAR (bf16), no PE involvement;
                # emitted after the K dup so it isn't stuck behind the
                # transposes in the Sync queue
                for i in range(4 * j, 4 * j + 4):
                    nc.sync.dma_start_transpose(
                        out=va[i][:, 0:64],
                        in_=vstage[:, i * SB:(i + 1) * SB])
                # ---- q projection + RoPE (pair m: heads 2m, 2m+1) ----
                for m in range(2):
                    pq = ps_proj.tile([128, NT], F32, tag="proj")
                    for ck in range(nck):
                        nc.tensor.matmul(
                            pq[:], wq_sb[:, ck, m * 128:(m + 1) * 128],
                            xt[j][:, ck, :],
                            start=(ck == 0), stop=(ck == nck - 1))
                    qtile = sb_qt.tile([128, NT], BF16, tag="qt")
                    qt[m][j] = qtile
                    tmp = sb_tmp.tile([128, NT], BF16, tag="rope_tmp")
                    for r in range(0, 128, 64):
                        nc.vector.tensor_mul(tmp[r:r + 32, :],
                                             pq[r + 32:r + 64, :],
                                             sin_sb[r:r + 32, jt])
                        nc.vector.tensor_mul(tmp[r + 32:r + 64, :],
                                             pq[r:r + 32, :],
                                             sin_sb[r + 32:r + 64, jt])
                    tmp2 = sb_tmp.tile([128, NT], BF16, tag="rope_tmp2")
                    nc.vector.tensor_mul(tmp2[:], pq[:], cos_sb[:, jt])
                    nc.gpsimd.tensor_add(qtile[:], tmp2[:], tmp[:])

            def wo_chunk(j, sc_i):
                # one 128-row slice of out_part for q tile j
                sl = slice((sc_i % 4) * 128, (sc_i % 4 + 1) * 128)
                osl = slice(sc_i * 128, (sc_i + 1) * 128)
                ost = sb_out.tile([128, D], BF16, tag="ost")
                for e in range(D // NT):
                    et = slice(e * NT, (e + 1) * NT)
                    po = ps_proj.tile([128, NT], F32, tag="proj")
                    for ck in range(2):
                        nc.tensor.matmul(po[:], at[ck][j][:, sl],
                                         wo_sb[:, ck, et],
                                         start=(ck == 0), stop=(ck == 1))
                    nc.vector.tensor_copy(ost[:, et], po[:])
                nc.sync.dma_start(out=out_part[osl, :], in_=ost[:])

            def attn_pair(j, p, wo_tiles):
                """Attention for q-tile j, head pair p, with w_o chunks of a
                previous tile interleaved to keep the PE fed."""
                nblk = 4 * j + 4
                pv_e = ps_pv.tile([65, NT], F32, tag="pv")
                pv_o = ps_pv.tile([65, NT], F32, tag="pv")
                # allocate the broadcast tile up front: its pool slot then
                # rotates ahead of the w_o po tiles, so the next projection
                # never waits on this pair's normalization tail
                bcp = ps_proj.tile([128, NT], F32, tag="proj")
                wo_every = max(1, nblk // max(1, len(wo_tiles)) + 1)
                wo_iter = list(wo_tiles)
                for i in range(nblk):
                    d = i - 4 * j          # >= 0: diagonal block
                    lo = 128 * d if d > 0 else 0
                    kb = kt[i // 4]
                    kc = slice((i % 4) * SB, (i % 4 + 1) * SB)
                    sc2 = ps_sc.tile([128, 2, NT], F32, tag="sc")
                    nc.tensor.matmul(sc2[:, 0, lo:], kb[0:64, kc],
                                     qt[p][j][0:64, lo:],
                                     start=True, stop=True,
                                     tile_position=(0, 0))
                    nc.tensor.matmul(sc2[:, 1, lo:], kb[64:128, kc],
                                     qt[p][j][64:128, lo:],
                                     start=True, stop=True,
                                     tile_position=(64, 0))
                    ex2 = sb_ex.tile([128, 2, NT], BF16, tag="ex")
                    nc.scalar.activation(ex2[:, :, lo:], sc2[:, :, lo:], EXP)
                    if d >= 0:
                        # zero the s > q corner of the diagonal block (both
                        # head halves in one call)
                        nc.gpsimd.affine_select(
                            out=ex2[:, :, lo:lo + 128],
                            in_=ex2[:, :, lo:lo + 128],
                            compare_op=mybir.AluOpType.is_ge, fill=0.0,
                            base=0, pattern=[[0, 2], [1, 128]],
                            channel_multiplier=-1)
                    nc.tensor.matmul(pv_e[:, lo:], va[i][:], ex2[:, 0, lo:],
                                     start=(i == 0), stop=(i == nblk - 1),
                                     skip_group_check=True)
                    nc.tensor.matmul(pv_o[:, lo:], va[i][:], ex2[:, 1, lo:],
                                     start=(i == 0), stop=(i == nblk - 1),
                                     skip_group_check=True)
                    if wo_iter and (i + 1) % wo_every == 0:
                        wo_chunk(*wo_iter.pop(0))
                for w in wo_iter:
                    wo_chunk(*w)
                # ---- softmax normalization, wide ops only ----
                # approx reciprocals of the two sum rows straight from PSUM
                # into partitions 0/64 of the persistent staging tile (rows
                # 1:64 hold 1.0), then one K=65 masked matmul broadcasts both
                # rows across partitions, and two DVE muls normalize.
                nc.vector.tensor_copy(sr[0:1, :], pv_e[64:65, :])
                nc.vector.tensor_copy(sr[64:65, :], pv_o[64:65, :])
                nc.vector.reciprocal_approx_fast(out=sr[:], in_=sr[:])
                nc.vector.tensor_copy(srr[:], sr[:])
                nc.tensor.matmul(bcp[:], mask65[:], srr[:],
                                 start=True, stop=True)
                # TT ops may read at most one PSUM operand: stage the
                # broadcast in SBUF before the two normalization muls
                brec = sb_tmp.tile([128, NT], F32, tag="brec")
                nc.vector.tensor_copy(brec[:], bcp[:])
                atile = sb_at.tile([128, NT], BF16, tag="at")
                at[p][j] = atile
                nc.vector.tensor_mul(atile[0:64, :], pv_e[0:64, :],
                                     brec[0:64, :])
                nc.vector.tensor_mul(atile[64:128, :], pv_o[0:64, :],
                                     brec[64:128, :])

            # Software pipeline: projections run one q-tile ahead so their
            # RoPE (DVE) overlaps the previous tile's ACT-bound attention,
            # and w_o of tile j-1 interleaves into attention of tile j to
            # fill the PE while ACT works through the exp backlog.
            proj_j(0)
            for j in range(nj):
                if j + 1 < nj:
                    proj_j(j + 1)
                if j == 0:
                    attn_pair(j, 0, [])
                    attn_pair(j, 1, [])
                else:
                    attn_pair(j, 0, [(j - 1, 4 * (j - 1) + k) for k in (0, 1)])
                    attn_pair(j, 1, [(j - 1, 4 * (j - 1) + k) for k in (2, 3)])
            for k in range(4):
                wo_chunk(nj - 1, 4 * (nj - 1) + k)

    nc.compile()
    return nc


def make_tables(S=S_FULL):
    half = DK // 2
    inv_freq = 1.0 / (ROPE_BASE ** (np.arange(half, dtype=np.float32) / half))
    t = np.arange(S, dtype=np.float32)
    freqs = np.outer(t, inv_freq)                      # [S, half]
    cosT = np.cos(freqs).T.astype(np.float32)          # [half, S]
    sinT = np.sin(freqs).T.astype(np.float32)
    cos4 = np.tile(cosT, (4, 1))                       # [128, S]
    sin4 = np.tile(np.concatenate([-sinT, sinT], axis=0), (2, 1))
    return np.ascontiguousarray(cos4), np.ascontiguousarray(sin4)


def make_in_maps(x, wq, wk, wv, wo, S=S_FULL):
    from ml_dtypes import bfloat16
    cos4, sin4 = make_tables(S)
    scale = 1.0 / np.sqrt(np.float32(DK))
    xTb = [np.ascontiguousarray(x[b].T).astype(bfloat16) for b in range(x.shape[0])]
    in_maps = []
    for c in range(N_CORES):
        b, kv = c // NKV, c % NKV
        wq_c = np.ascontiguousarray(
            wq[:, kv * 256:(kv + 1) * 256] * scale).astype(bfloat16)
        wkv_c = np.ascontiguousarray(np.concatenate(
            [wv[:, kv * DK:(kv + 1) * DK], wk[:, kv * DK:(kv + 1) * DK]],
            axis=1)).astype(bfloat16)
        wo_c = np.ascontiguousarray(wo[kv * 256:(kv + 1) * 256, :]).astype(
            bfloat16)
        in_maps.append({
            "xT": xTb[b], "wq": wq_c, "wkv": wkv_c, "wo": wo_c,
            "cos4": cos4, "sin4": sin4,
        })
    return in_maps


_NC_CACHE = {}


def kernel(x, wq, wk, wv, wo, _trace=False):
    x = np.asarray(x, dtype=np.float32)
    S = x.shape[1]
    if S not in _NC_CACHE:
        _NC_CACHE[S] = build_nc(S)
    nc = _NC_CACHE[S]
    in_maps = make_in_maps(x, np.asarray(wq, np.float32),
                           np.asarray(wk, np.float32),
                           np.asarray(wv, np.float32),
                           np.asarray(wo, np.float32), S)
    res = run_bass_kernel_spmd(nc, in_maps, list(range(N_CORES)),
                               trace=_trace)
    kernel.last_result = res
    out = np.zeros((x.shape[0], S, D), dtype=np.float32)
    for c in range(N_CORES):
        out[c // NKV] += res.results[c]["out_part"].astype(np.float32)
    return out
